# revision 31
# baseline (speedup 1.0000x reference)
"""Trainium2 Bass kernel for MultiHeadedAttention with RoPE (v3).

Problem: b=4, n=2048, d=1024, H=16 heads, dk=64, rotary on first 32 dims
(interleaved pairs, theta=10000, lucidrains convention).

Sharding: 8 cores = 4 batches x 2 query-halves (data parallel). Each core
computes the full K/V projections for its batch (replicated across the 2
query-half siblings) and attention + output projection for its 1024 query
rows. No collectives needed; host gathers/concatenates.

Device-side layout strategy (all "transposed", features on partitions):
  - Host passes X.T (d-major) so projections need no on-device transposes.
  - RoPE: host permutes Wq/Wk output features per head to [evens(16),
    pass(16), odds(16), pass(16)] so the interleaved pair rotation becomes
    a +-32 partition-offset multiply against cos/sin tables.
  - scores.T = K_rot.T' @ Q_rot.T per head; the two heads of a
    128-partition chunk run as concurrent row-group matmuls (row tiling).
  - softmax: exp without max-subtraction (scores O(1)); normalizer Z from
    a ones-column appended to V; 1/Z broadcast across partitions via
    gpsimd partition_broadcast.

v3 structural changes vs v2 (all driven by HW microbenchmarks — the
CoreSim cost model does not model LDWEIGHTS or PSUM-accumulate stalls):
  - attnV psum accumulation rotated across banks: same-bank accumulating
    matmuls closer than ~850ns stall the PE (and trip HAM re-throttle).
    h0 rotates over 2 banks (merged on DVE at normalize), h1 stays
    single-banked (its WAW distance is ACT-paced, which is enough).
  - K projection no longer emitted as contiguous 4-matmul accumulation
    chains (WAW-1): split into single-matmul pieces spread one per kc
    iteration of the attention loop.
  - Q projection halves interleaved (WAW-2 at DoubleRow spacing is safe).
  - rope computed with plain bf16 tensor_tensor muls (2x DVE mode) after
    folding the bias into the psum eviction (tensor_scalar_add); the v2
    scalar_tensor_tensor ops ran at 1x.
  - V' bias+ones folded into a DVE eviction add against a host-broadcast
    bias tile (kills the K=1 bias matmuls and keeps ACT exp-only).
  - out projection: 8 rotating psum banks (4 dmc x 2 rn), WAW distance 8.
"""

import os

import numpy as np

B, N, D = 4, 2048, 1024
H, DK = 16, 64
ROT, HALF = 32, 16
THETA = 10000.0
NCORES = 8
NQ = N // 2  # query rows per core

_PROGRAM_CACHE = {}


def _build_program(mm_dtype_name="bfloat16"):
    import concourse.tile as tile
    from concourse import bacc, mybir
    from contextlib import ExitStack

    PHASES = int(os.environ.get("KPHASES", "9"))  # debug bisect knob
    KLOOP = int(os.environ.get("KLOOP", "1"))      # hw-loop repeat (timing)
    SKIPEXP = os.environ.get("KSKIPEXP", "0") == "1"    # timing-only bisect
    SKIPPROJ = os.environ.get("KSKIPPROJ", "0") == "1"  # timing-only bisect

    f32 = mybir.dt.float32
    mmdt = getattr(mybir.dt, mm_dtype_name)
    AF = mybir.ActivationFunctionType

    NFC_ = D // 128
    nc = bacc.Bacc("TRN2", target_bir_lowering=False)

    # DRAM I/O (per core). All *T tensors are feature-major (transposed).
    f8 = mybir.dt.float8e4
    xq8T = nc.dram_tensor("xq8T", [D // 2, 2, NQ], f8, kind="ExternalInput")
    xk8T = nc.dram_tensor("xk8T", [D // 2, 2, N], f8, kind="ExternalInput")
    xvT = nc.dram_tensor("xvT", [D, N], mmdt, kind="ExternalInput")
    wq8T = nc.dram_tensor("wq8T", [128, NFC_, 1024], f8, kind="ExternalInput")
    wk8T = nc.dram_tensor("wk8T", [128, NFC_, 1024], f8, kind="ExternalInput")
    wvT = nc.dram_tensor("wvT", [D, H * 65], mmdt, kind="ExternalInput")
    wvb = nc.dram_tensor("wvb", [128, H * 65], mmdt, kind="ExternalInput")
    woT = nc.dram_tensor("woT", [D, D], mmdt, kind="ExternalInput")
    bq_d = nc.dram_tensor("bq_d", [D], f32, kind="ExternalInput")
    bk_d = nc.dram_tensor("bk_d", [D], f32, kind="ExternalInput")
    bo_d = nc.dram_tensor("bo_d", [D], f32, kind="ExternalInput")
    cosQ = nc.dram_tensor("cosQ", [128, NQ], mmdt, kind="ExternalInput")
    sinQ = nc.dram_tensor("sinQ", [128, NQ], mmdt, kind="ExternalInput")
    cosK = nc.dram_tensor("cosK", [128, N], mmdt, kind="ExternalInput")
    sinK = nc.dram_tensor("sinK", [128, N], mmdt, kind="ExternalInput")
    outT = nc.dram_tensor("outT", [D, NQ], mmdt, kind="ExternalOutput")

    NKC = N // 128       # 16 key chunks
    NFC = D // 128       # 8 feature chunks
    NVC = D // 128       # 8 contraction chunks for V

    with ExitStack() as ctx:
        tc = ctx.enter_context(tile.TileContext(nc))

        const = ctx.enter_context(tc.tile_pool(name="const", bufs=1))

        # persistent sbuf tensors
        v_sb = const.tile([128, NKC, H * 65], mmdt)   # V' (keys, per-head 64+ones)
        q_sb = const.tile([128, NFC, NQ], mmdt)       # Q_rot.T
        k_sb = const.tile([128, NFC, N], mmdt)        # K_rot.T
        y_sb = const.tile([128, NFC, NQ], mmdt)       # Y.T (normalized attn out)
        bq_sb = const.tile([128, NFC], f32)
        bk_sb = const.tile([128, NFC], f32)
        bo_sb = const.tile([128, NFC], f32)
        ck_sb = const.tile([128, N], mmdt)
        sk_sb = const.tile([128, N], mmdt)
        cq_sb = const.tile([128, NQ], mmdt)
        sq_sb = const.tile([128, NQ], mmdt)
        wvb_sb = const.tile([128, H * 65], mmdt)
        xq_sb = const.tile([128, NFC // 2, 2, NQ], f8)

        _dmaq = [nc.sync, nc.scalar, nc.gpsimd]
        _dmaqi = [0]

        def dma_rr(dst, src_ap):
            eng = _dmaq[_dmaqi[0] % len(_dmaq)]
            _dmaqi[0] += 1
            eng.dma_start(dst, src_ap)

        def load_chunked(dst_tile, src_t, nchunks, splits=4):
            per = nchunks // splits if nchunks % splits == 0 else 1
            if per == 0:
                per = 1
            c = 0
            while c < nchunks:
                n = min(per, nchunks - c)
                dma_rr(
                    dst_tile[:, c:c + n, :],
                    src_t[c * 128:(c + n) * 128, :].rearrange(
                        "(c p) r -> p c r", p=128),
                )
                c += n

        def load_consts():
            nc.sync.dma_start(bq_sb[:], bq_d.rearrange("(c p) -> p c", p=128))
            nc.sync.dma_start(bk_sb[:], bk_d.rearrange("(c p) -> p c", p=128))
            nc.sync.dma_start(bo_sb[:], bo_d.rearrange("(c p) -> p c", p=128))
            nc.sync.dma_start(wvb_sb[:], wvb[:])
            nc.gpsimd.dma_start(ck_sb[:], cosK[:])
            nc.gpsimd.dma_start(sk_sb[:], sinK[:])
            nc.scalar.dma_start(cq_sb[:], cosQ[:])
            nc.scalar.dma_start(sq_sb[:], sinQ[:])

        rope_pool = ctx.enter_context(tc.tile_pool(name="rope", bufs=3))

        def load_pairs(dst_tile, src_t, rows):
            for c in range(NFC // 2):
                dma_rr(
                    dst_tile[:, c, :, :],
                    src_t[c * 128:(c + 1) * 128, :, :],
                )

        # Q/K weight column staging (per output-feature chunk, 2 in flight)
        wqp = ctx.enter_context(tc.tile_pool(name="wqstage", bufs=2))
        wkp = ctx.enter_context(tc.tile_pool(name="wkstage", bufs=2))
        _staged_wq = {}
        _staged_wk = {}

        def stage_wq(fc):
            wq = wqp.tile([128, NFC // 2, 2, 128], f8, tag="wq",
                          name=f"wq{fc}")
            dma_rr(wq[:, :, :, :], wq8T[:, fc, :])
            _staged_wq[fc] = wq
            return wq

        def stage_wk(fc):
            wk = wkp.tile([128, NFC // 2, 2, 128], f8, tag="wk",
                          name=f"wk{fc}")
            dma_rr(wk[:, :, :, :], wk8T[:, fc, :])
            _staged_wk[fc] = wk
            return wk

        def get_staged_wq(fc):
            return _staged_wq.pop(fc) if fc in _staged_wq else stage_wq(fc)

        def get_staged_wk(fc):
            return _staged_wk.pop(fc) if fc in _staged_wk else stage_wk(fc)

        # Persistent zeroed sin-term temporaries: pass rows stay zero forever;
        # only the 4x16 rotary rows are rewritten each block.
        tmpS_tiles = [
            const.tile([128, 1024], mmdt, tag=f"tmpS{i}", name=f"tmpS{i}")
            for i in (0, 1)
        ]
        for t in tmpS_tiles:
            nc.vector.memset(t[:], 0.0)
        _blk = [0]

        # Per-head feature layout (after the host permutation):
        #   [0:16) evens, [16:32) pass, [32:48) odds, [48:64) pass
        # so rotary partners are at +-32 partitions (quadrant aligned).
        # pc already carries the bias (folded into the psum eviction), so
        # every rope op is a plain bf16 tensor_tensor (2x DVE mode).
        def rope_1024(pc, cos_sb, sin_sb, dst_ap, r0):
            blk = _blk[0]
            _blk[0] += 1
            tmpC = rope_pool.tile([128, 1024], mmdt, tag="tmpC",
                                  name=f"tmpC{blk}")
            tmpS = tmpS_tiles[blk % 2]
            nc.vector.tensor_mul(tmpC[:], pc[:], cos_sb[:, r0:r0 + 1024])
            # sin part: out rows R read pc at the partner rows P = R +- 32;
            # the sin table is negated so the value at the partner location
            # is the sign-correct coefficient for row R.
            for h2 in (0, 64):
                nc.vector.tensor_mul(
                    tmpS[h2:h2 + 16, :],
                    pc[h2 + 32:h2 + 48, :],
                    sin_sb[h2 + 32:h2 + 48, r0:r0 + 1024],
                )
                nc.vector.tensor_mul(
                    tmpS[h2 + 32:h2 + 48, :],
                    pc[h2:h2 + 16, :],
                    sin_sb[h2:h2 + 16, r0:r0 + 1024],
                )
            nc.vector.tensor_add(dst_ap, tmpC[:], tmpS[:])

        def phase_v():
            with tc.tile_pool(name="vphase", bufs=1) as vp, \
                 tc.tile_pool(name="vpsum", bufs=8, space="PSUM") as vps:
                xv_sb = vp.tile([128, NVC, N], mmdt)
                wv_sb = vp.tile([128, NVC, H * 65], mmdt)
                for dc in range(NVC):
                    dma_rr(xv_sb[:, dc, :],
                           xvT[dc * 128:(dc + 1) * 128, :])
                    dma_rr(wv_sb[:, dc, :],
                           wvT[dc * 128:(dc + 1) * 128, :])
                load_consts()
                # prefetch next phase's input + first weight stages
                load_pairs(xq_sb, xq8T, NQ)
                stage_wq(0)
                stage_wk(0)
                for kc in range(NKC):
                    # the bufs=8 ring gives consecutive kc disjoint bank
                    # quads: per-bank accumulate WAW distance is 8 matmuls
                    # (~870ns), clear of the psum-accumulate stall.
                    pss = [vps.tile([128, 260], f32, tag="vps",
                                    name=f"vps{kc}_{i}")
                           for i in range(4)]
                    for dc in range(NVC):
                        for nf in range(4):
                            nc.tensor.matmul(
                                pss[nf][:],
                                lhsT=xv_sb[:, dc, kc * 128:(kc + 1) * 128],
                                rhs=wv_sb[:, dc, nf * 260:(nf + 1) * 260],
                                start=(dc == 0),
                                stop=(dc == NVC - 1),
                            )
                    # bias + ones column folded into the DVE eviction add
                    # against the host-broadcast wvb tile.
                    for nf in range(4):
                        nc.vector.tensor_add(
                            v_sb[:, kc, nf * 260:(nf + 1) * 260], pss[nf][:],
                            wvb_sb[:, nf * 260:(nf + 1) * 260],
                        )

        def make_proj_pieces(fc, w_sb, x_sb, cos_sb, sin_sb, b_sb, dst,
                             ncols, qk_psum):
            """Q or K projection for one feature chunk as a list of small
            closures (mostly one matmul each) to be spread across the kc
            iterations of the attention loop: same-bank accumulating
            matmuls land ~1us apart (no psum-accumulate stall) and the PE
            never waits long on an eviction."""
            pieces = []
            for pairi in range(ncols // 1024):
                pc = [None]
                ps = [None, None]

                def start_block(pairi=pairi, pc=pc, ps=ps):
                    pc[0] = rope_pool.tile([128, 1024], mmdt, tag="pscopy",
                                           name=f"pc{fc}_{pairi}")
                    for sub in range(2):
                        ps[sub] = qk_psum.tile([128, 512], f32, tag="qkps",
                                               name=f"qkps{fc}_{pairi}_{sub}")

                pieces.append(start_block)
                # dcp-outer: each DoubleRow stationary is reused by the two
                # halves' matmuls (one LDWEIGHTS per 2 matmuls); the halves'
                # accumulation chains interleave, and pieces are spread ~1
                # per kc so per-bank WAW distance stays >1us.
                for dcp in range(NFC // 2):
                    for sub in range(2):
                        half = pairi * 2 + sub

                        def mm(dcp=dcp, half=half, sub=sub, ps=ps):
                            nc.tensor.matmul(
                                ps[sub][:],
                                lhsT=w_sb[:, dcp, :, :],
                                rhs=x_sb[:, dcp, :,
                                         half * 512:(half + 1) * 512],
                                start=(dcp == 0),
                                stop=(dcp == NFC // 2 - 1),
                                perf_mode=mybir.MatmulPerfMode.DoubleRow,
                            )
                        pieces.append(mm)

                for sub in range(2):
                    def evict(sub=sub, ps=ps, pc=pc):
                        # on ACT: the next block's start=True matmul WAW-waits
                        # this eviction; ACT's lag is bounded (~1 exp), while
                        # the DVE backlog would stall the in-order PE queue.
                        nc.scalar.activation(
                            pc[0][:, sub * 512:(sub + 1) * 512], ps[sub][:],
                            AF.Identity, bias=b_sb[:, fc:fc + 1])
                    pieces.append(evict)

                def rope_fin(pairi=pairi, pc=pc):
                    r0 = pairi * 1024
                    rope_1024(pc[0], cos_sb, sin_sb,
                              dst[:, fc, r0:r0 + 1024], r0)
                pieces.append(rope_fin)
            return pieces

        def make_qk_pieces(fc, xk_sb, qk_psum):
            wq = get_staged_wq(fc)
            wk = get_staged_wk(fc)
            return (
                make_proj_pieces(fc, wq, xq_sb, cq_sb, sq_sb, bq_sb, q_sb,
                                 NQ, qk_psum)
                + make_proj_pieces(fc, wk, xk_sb, ck_sb, sk_sb, bk_sb, k_sb,
                                   N, qk_psum)
            )

        def attn_pair(fc, spool, poApool, po1pool, ppool, npool,
                      pieces):
            """Attention for heads (2*fc, 2*fc+1), two query-half passes.
            h0 accumulates into 2 rotating psum banks (merged on DVE);
            h1 single-banked (ACT-paced WAW spacing is sufficient).
            pieces (next pair's K projection) spread into the kc loop."""
            h0, h1 = 2 * fc, 2 * fc + 1

            for qn in range(2):
                po0 = poApool.tile([65, 512], f32, tag="poA",
                                   name=f"poA{fc}_{qn}")
                po1 = po1pool.tile([65, 512], f32, tag="po1",
                                   name=f"po1{fc}_{qn}")
                pts = {}

                def scores_kc(kc):
                    st = spool.tile([128, 2, 512], f32, tag="st",
                                    name=f"st{fc}_{qn}_{kc}")
                    for i, hb in enumerate((0, 64)):
                        nc.tensor.matmul(
                            st[:, i, :],
                            lhsT=k_sb[hb:hb + 64, fc, kc * 128:(kc + 1) * 128],
                            rhs=q_sb[hb:hb + 64, fc,
                                     qn * 512:(qn + 1) * 512],
                            start=True,
                            stop=True,
                        )
                    return st

                def exp_kc(kc, st):
                    pt = ppool.tile([128, 2, 512], mmdt, tag="pt",
                                    name=f"pt{fc}_{qn}_{kc}")
                    if SKIPEXP:
                        if kc == 0:
                            nc.vector.memset(pt[:], 0.01)
                            pts["static"] = pt
                        pt = pts["static"]
                    else:
                        nc.scalar.activation(pt[:], st[:], AF.Exp,
                                             scale=1.0 / (8.0 * 1024.0))
                    pts[kc] = pt

                def attnv_kc(kc):
                    # single accumulator bank per head: the attnV pair for
                    # kc lands ~1.1us (one ACT exp) after kc-1's, clear of
                    # the same-bank psum-accumulate stall.
                    nc.tensor.matmul(
                        po0[:],
                        lhsT=v_sb[:, kc, h0 * 65:(h0 + 1) * 65],
                        rhs=pts[kc][:, 0, :],
                        start=(kc == 0),
                        stop=(kc == NKC - 1),
                    )
                    nc.tensor.matmul(
                        po1[:],
                        lhsT=v_sb[:, kc, h1 * 65:(h1 + 1) * 65],
                        rhs=pts[kc][:, 1, :],
                        start=(kc == 0),
                        stop=(kc == NKC - 1),
                    )
                    del pts[kc]

                # software pipeline: scores(kc) || exp(kc) || attnV(kc-1),
                # K-projection pieces spread between iterations (front-
                # loaded so the DVE is free near the group boundary)
                st = scores_kc(0)
                exp_kc(0, st)
                for kc in range(1, NKC):
                    st = scores_kc(kc)
                    attnv_kc(kc - 1)
                    exp_kc(kc, st)
                    # 42 pieces spread over the 30 kc-slots of the 2 qn
                    # passes, front-loaded within each pass
                    npop = 0 if SKIPPROJ else (2 if kc <= 11 else 1)
                    for _ in range(npop):
                        if pieces:
                            pieces.pop(0)()
                attnv_kc(NKC - 1)

                # psum-freeing copies on ACT (bounded lag): the next group's
                # attnV start=True matmuls WAW-wait these reads.
                ya = npool.tile([65, 512], f32, tag="ya",
                                name=f"ya{fc}_{qn}")
                nc.scalar.activation(ya[:], po1[:], AF.Identity)
                yb = npool.tile([65, 512], f32, tag="yb",
                                name=f"yb{fc}_{qn}")
                nc.scalar.activation(yb[:], po0[:], AF.Identity)
                for i, (yy, hb) in enumerate(((yb, 0), (ya, 64))):
                    rz = npool.tile([1, 512], f32, tag=f"rz{i}", bufs=1,
                                    name=f"rz{i}_{fc}_{qn}")
                    nc.vector.reciprocal(rz[:], yy[64:65, :])
                    rzb = npool.tile([64, 512], f32, tag=f"rzb{i}", bufs=1,
                                     name=f"rzb{i}_{fc}_{qn}")
                    nc.gpsimd.partition_broadcast(rzb[:], rz[:])
                    nc.vector.tensor_mul(
                        y_sb[hb:hb + 64, fc, qn * 512:(qn + 1) * 512],
                        yy[0:64, :], rzb[:]
                    )

        def phase_qk_attn(do_attn):
            with tc.tile_pool(name="kstage", bufs=1) as kp:
                xk_sb = kp.tile([128, NFC // 2, 2, N], f8)
                wo_sb = kp.tile([128, NFC, D], mmdt)
                load_pairs(xk_sb, xk8T, N)
                # prefetch output-projection weights into the attn window
                load_chunked(wo_sb, woT, NFC)
                phase_qk_attn_inner(do_attn, xk_sb)
                # attn psum pools are closed; out projection runs here so
                # wo_sb (kstage) stays alive
                if PHASES >= 5:
                    phase_out(wo_sb)

        def phase_qk_attn_inner(do_attn, xk_sb):
            with tc.tile_pool(name="qkpsum", bufs=2, space="PSUM") as qk_psum:
                if do_attn:
                    with tc.tile_pool(name="spsum", bufs=2,
                                      space="PSUM") as spool, \
                         tc.tile_pool(name="poApsum", bufs=1,
                                      space="PSUM") as poApool, \
                         tc.tile_pool(name="po1psum", bufs=1,
                                      space="PSUM") as po1pool, \
                         tc.tile_pool(name="ptpool", bufs=5) as ppool, \
                         tc.tile_pool(name="npool", bufs=2) as npool:
                        # pair 0's Q+K projections run dense; pair p's attn
                        # spreads pair p+1's projection pieces.
                        for piece in make_qk_pieces(0, xk_sb, qk_psum):
                            piece()
                        for fc in range(NFC):
                            if fc + 1 < NFC:
                                pieces = make_qk_pieces(fc + 1, xk_sb,
                                                        qk_psum)
                            else:
                                pieces = []
                            attn_pair(fc, spool, poApool, po1pool,
                                      ppool, npool, pieces)
                            assert SKIPPROJ or not pieces, \
                                f"{len(pieces)} pieces left"
                else:
                    for fc in range(NFC):
                        for piece in make_qk_pieces(fc, xk_sb, qk_psum):
                            piece()
                    nc.vector.memset(y_sb[:], 0.0)

        def phase_out(wo_sb):
            # 4 dmc x 2 rn rotating accumulator banks: per-bank WAW
            # distance is 8 matmuls, no accumulate stall.
            with tc.tile_pool(name="owork", bufs=3) as owork, \
                 tc.tile_pool(name="opsum2", bufs=8, space="PSUM") as ops2:
                for quad in range(2):
                    pss = {}
                    for d4 in range(4):
                        for rn in range(2):
                            pss[(d4, rn)] = ops2.tile(
                                [128, 512], f32, tag="ops",
                                name=f"ops{quad}_{d4}_{rn}")
                    for fc in range(NFC):
                        for d4 in range(4):
                            dmc = quad * 4 + d4
                            for rn in range(2):
                                nc.tensor.matmul(
                                    pss[(d4, rn)][:],
                                    lhsT=wo_sb[:, fc,
                                               dmc * 128:(dmc + 1) * 128],
                                    rhs=y_sb[:, fc, rn * 512:(rn + 1) * 512],
                                    start=(fc == 0),
                                    stop=(fc == NFC - 1),
                                )
                    for d4 in range(4):
                        dmc = quad * 4 + d4
                        ob = owork.tile([128, NQ], mmdt, tag="ob",
                                        name=f"ob{quad}_{d4}")
                        for rn in range(2):
                            nc.vector.tensor_scalar_add(
                                ob[:, rn * 512:(rn + 1) * 512],
                                pss[(d4, rn)][:],
                                bo_sb[:, dmc:dmc + 1])
                        eng = nc.sync if d4 % 2 == 0 else nc.gpsimd
                        eng.dma_start(outT[dmc * 128:(dmc + 1) * 128, :],
                                      ob[:])

        def all_phases():
            if PHASES >= 1:
                phase_v()
            if PHASES >= 2:
                phase_qk_attn(do_attn=(PHASES >= 4))
            else:
                nc.vector.memset(y_sb[:], 0.0)
            if PHASES < 5:
                with tc.tile_pool(name="dummy", bufs=1) as dp:
                    zb = dp.tile([128, NQ], mmdt)
                    nc.vector.memset(zb[:], 0.0)
                    for dmc in range(NFC):
                        nc.sync.dma_start(outT[dmc * 128:(dmc + 1) * 128, :],
                                          zb[:])

        if KLOOP > 1:
            with tc.For_i(0, KLOOP, 1,
                          hint_engines=(mybir.EngineType.PE,
                                        mybir.EngineType.DVE,
                                        mybir.EngineType.Activation),
                          staggered_reset=True):
                all_phases()
        else:
            all_phases()

    nc.compile()
    return nc


def _rope_tables(positions):
    """cos/sin tables [128, len(positions)] for the permuted transposed
    layout: partition p (within a 2-head feature chunk), j = p % 64:
    j<16: freq j (cos, -sin); 32<=j<48: freq j-32 (cos, +sin); else (1, 0)."""
    inv_freq = 1.0 / (THETA ** (np.arange(0, ROT, 2, dtype=np.float64) / ROT))  # [16]
    t = np.asarray(positions, dtype=np.float64)
    ang = t[None, :] * inv_freq[:, None]  # [16, nt]
    c, s = np.cos(ang), np.sin(ang)
    cos_tab = np.ones((128, len(positions)), dtype=np.float64)
    sin_tab = np.zeros((128, len(positions)), dtype=np.float64)
    for h2 in (0, 64):
        cos_tab[h2:h2 + 16] = c
        cos_tab[h2 + 32:h2 + 48] = c
        # NEGATED sin coefficients (+s for evens, -s for odds): the device
        # reads the sin operand at the +-32 partner partition, where the
        # negated opposite-sign value equals the row's own coefficient.
        sin_tab[h2:h2 + 16] = s
        sin_tab[h2 + 32:h2 + 48] = -s
    return cos_tab.astype(np.float32), sin_tab.astype(np.float32)


def _head_perm():
    """Feature permutation applied to rows of Wq/Wk (and bq/bk): within each
    head's 64 outputs -> [evens(16), pass 32:48, odds(16), pass 48:64]."""
    out = np.empty(D, dtype=np.int64)
    for h in range(H):
        base = h * DK
        out[base:base + HALF] = base + np.arange(0, ROT, 2)
        out[base + HALF:base + ROT] = base + np.arange(ROT, ROT + HALF)
        out[base + ROT:base + ROT + HALF] = base + np.arange(1, ROT, 2)
        out[base + ROT + HALF:base + DK] = base + np.arange(ROT + HALF, DK)
    return out


def _prep_inputs(query, key, value, Wq, bq, Wk, bk, Wv, bv, Wo, bo,
                 mm_dtype_name="bfloat16"):
    import ml_dtypes

    np_mm = ml_dtypes.bfloat16 if mm_dtype_name == "bfloat16" else np.float32

    query = np.asarray(query, np.float32)
    key = np.asarray(key, np.float32)
    value = np.asarray(value, np.float32)
    Wq, bq = np.asarray(Wq, np.float32), np.asarray(bq, np.float32)
    Wk, bk = np.asarray(Wk, np.float32), np.asarray(bk, np.float32)
    Wv, bv = np.asarray(Wv, np.float32), np.asarray(bv, np.float32)
    Wo, bo = np.asarray(Wo, np.float32), np.asarray(bo, np.float32)

    perm = _head_perm()
    Wq_p, bq_p = Wq[perm], bq[perm]
    Wk_p, bk_p = Wk[perm], bk[perm]
    np_f8 = ml_dtypes.float8_e4m3

    def _pairs(a):  # [D, cols] -> [D//2, 2, cols] DoubleRow interleave
        d, cols = a.shape
        out = np.empty((d // 2, 2, cols), a.dtype)
        for dcp in range(d // 256):
            for j in (0, 1):
                out[dcp * 128:(dcp + 1) * 128, j, :] = \
                    a[(2 * dcp + j) * 128:(2 * dcp + j + 1) * 128, :]
        return out

    def _wflat(wt):  # [D, D] lhsT -> [128, NFC, 1024] = [p, fc, (c j m)]
        t = wt.reshape(4, 2, 128, H // 2, 128)  # [c, j, p, fc, m]
        return np.ascontiguousarray(
            t.transpose(2, 3, 0, 1, 4).reshape(128, H // 2, 1024))

    # Q/K path in fp8: weights prescaled by 32 (folded out in the exp scale)
    wq8T = _wflat((Wq_p.T * 32.0).astype(np_f8))
    wk8T = _wflat((Wk_p.T * 32.0).astype(np_f8))
    bq_p = bq_p * 32.0
    bk_p = bk_p * 32.0
    woT = np.ascontiguousarray(Wo.T).astype(np_mm)

    # W_v' : [D, H*65] plus a broadcast bias/ones tile wvb [128, H*65]
    wvT = np.zeros((D, H * 65), np.float32)
    wvb = np.zeros((1, H * 65), np.float32)
    for h in range(H):
        cols = slice(h * 65, h * 65 + 64)
        wvT[:D, cols] = Wv[h * DK:(h + 1) * DK, :].T
        wvb[0, cols] = bv[h * DK:(h + 1) * DK]
        wvb[0, h * 65 + 64] = 1.0
    wvT = wvT.astype(np_mm)
    wvb_bc = np.ascontiguousarray(np.repeat(wvb, 128, axis=0)).astype(np_mm)

    cos_all, sin_all = _rope_tables(np.arange(N))

    in_maps = []
    for core in range(NCORES):
        b, qh = core // 2, core % 2
        rows = slice(qh * NQ, (qh + 1) * NQ)
        xq8T = np.ascontiguousarray(
            _pairs(query[b, rows, :].T.astype(np_f8)))
        xk8T = np.ascontiguousarray(_pairs(key[b].T.astype(np_f8)))
        xvT = np.ascontiguousarray(value[b].T).astype(np_mm)
        in_maps.append({
            "xq8T": xq8T,
            "xk8T": xk8T,
            "xvT": xvT,
            "wq8T": wq8T, "wk8T": wk8T, "wvT": wvT, "woT": woT,
            "wvb": wvb_bc,
            "bq_d": bq_p, "bk_d": bk_p, "bo_d": bo,
            "cosQ": np.ascontiguousarray(cos_all[:, rows]).astype(np_mm),
            "sinQ": np.ascontiguousarray(sin_all[:, rows]).astype(np_mm),
            "cosK": cos_all.astype(np_mm),
            "sinK": sin_all.astype(np_mm),
        })
    return in_maps


def kernel(query, key, value, Wq, bq, Wk, bk, Wv, bv, Wo, bo):
    from concourse import bass_utils

    mm_dtype_name = "bfloat16"
    if mm_dtype_name not in _PROGRAM_CACHE:
        _PROGRAM_CACHE[mm_dtype_name] = _build_program(mm_dtype_name)
    nc = _PROGRAM_CACHE[mm_dtype_name]

    in_maps = _prep_inputs(query, key, value, Wq, bq, Wk, bk, Wv, bv, Wo, bo,
                           mm_dtype_name)

    res = bass_utils.run_bass_kernel_spmd(
        nc, in_maps, core_ids=list(range(NCORES))
    )

    out = np.empty((B, N, D), np.float32)
    for core in range(NCORES):
        b, qh = core // 2, core % 2
        out[b, qh * NQ:(qh + 1) * NQ, :] = \
            res.results[core]["outT"].T.astype(np.float32)
    return out


# revision 32
# speedup vs baseline: 1.1254x; 1.1254x over previous
"""Trainium2 Bass kernel for MultiHeadedAttention with RoPE (v3).

Problem: b=4, n=2048, d=1024, H=16 heads, dk=64, rotary on first 32 dims
(interleaved pairs, theta=10000, lucidrains convention).

Sharding: 8 cores = 4 batches x 2 query-halves (data parallel). Each core
computes the full K/V projections for its batch (replicated across the 2
query-half siblings) and attention + output projection for its 1024 query
rows. No collectives needed; host gathers/concatenates.

Device-side layout strategy (all "transposed", features on partitions):
  - Host passes X.T (d-major) so projections need no on-device transposes.
  - RoPE: host permutes Wq/Wk output features per head to [evens(16),
    pass(16), odds(16), pass(16)] so the interleaved pair rotation becomes
    a +-32 partition-offset multiply against cos/sin tables.
  - scores.T = K_rot.T' @ Q_rot.T per head; the two heads of a
    128-partition chunk run as concurrent row-group matmuls (row tiling).
  - softmax: exp without max-subtraction (scores O(1)); normalizer Z from
    a ones-column appended to V; 1/Z broadcast across partitions via
    gpsimd partition_broadcast.

v3 structural changes vs v2 (all driven by HW microbenchmarks — the
CoreSim cost model does not model LDWEIGHTS or PSUM-accumulate stalls):
  - attnV psum accumulation rotated across banks: same-bank accumulating
    matmuls closer than ~850ns stall the PE (and trip HAM re-throttle).
    h0 rotates over 2 banks (merged on DVE at normalize), h1 stays
    single-banked (its WAW distance is ACT-paced, which is enough).
  - K projection no longer emitted as contiguous 4-matmul accumulation
    chains (WAW-1): split into single-matmul pieces spread one per kc
    iteration of the attention loop.
  - Q projection halves interleaved (WAW-2 at DoubleRow spacing is safe).
  - rope computed with plain bf16 tensor_tensor muls (2x DVE mode) after
    folding the bias into the psum eviction (tensor_scalar_add); the v2
    scalar_tensor_tensor ops ran at 1x.
  - V' bias+ones folded into a DVE eviction add against a host-broadcast
    bias tile (kills the K=1 bias matmuls and keeps ACT exp-only).
  - out projection: 8 rotating psum banks (4 dmc x 2 rn), WAW distance 8.
"""

import os

import numpy as np

B, N, D = 4, 2048, 1024
H, DK = 16, 64
ROT, HALF = 32, 16
THETA = 10000.0
NCORES = 8
NQ = N // 2  # query rows per core

_PROGRAM_CACHE = {}


def _build_program(mm_dtype_name="bfloat16"):
    import concourse.tile as tile
    from concourse import bacc, mybir
    from contextlib import ExitStack

    PHASES = int(os.environ.get("KPHASES", "9"))  # debug bisect knob
    KLOOP = int(os.environ.get("KLOOP", "1"))      # hw-loop repeat (timing)
    SKIPEXP = os.environ.get("KSKIPEXP", "0") == "1"    # timing-only bisect
    SKIPPROJ = os.environ.get("KSKIPPROJ", "0") == "1"  # timing-only bisect

    f32 = mybir.dt.float32
    mmdt = getattr(mybir.dt, mm_dtype_name)
    AF = mybir.ActivationFunctionType

    NFC_ = D // 128
    nc = bacc.Bacc("TRN2", target_bir_lowering=False)

    # DRAM I/O (per core). All *T tensors are feature-major (transposed).
    f8 = mybir.dt.float8e4
    xq8T = nc.dram_tensor("xq8T", [D // 2, 2, NQ], f8, kind="ExternalInput")
    xk8T = nc.dram_tensor("xk8T", [D // 2, 2, N], f8, kind="ExternalInput")
    xvT = nc.dram_tensor("xvT", [D, N], mmdt, kind="ExternalInput")
    wq8T = nc.dram_tensor("wq8T", [128, NFC_, 1024], f8, kind="ExternalInput")
    wk8T = nc.dram_tensor("wk8T", [128, NFC_, 1024], f8, kind="ExternalInput")
    wvT = nc.dram_tensor("wvT", [D, H * 65], mmdt, kind="ExternalInput")
    wvb = nc.dram_tensor("wvb", [128, H * 65], mmdt, kind="ExternalInput")
    woT = nc.dram_tensor("woT", [D, D], mmdt, kind="ExternalInput")
    bq_d = nc.dram_tensor("bq_d", [D], f32, kind="ExternalInput")
    bk_d = nc.dram_tensor("bk_d", [D], f32, kind="ExternalInput")
    bo_d = nc.dram_tensor("bo_d", [D], f32, kind="ExternalInput")
    cosQ = nc.dram_tensor("cosQ", [128, NQ], mmdt, kind="ExternalInput")
    sinQ = nc.dram_tensor("sinQ", [128, NQ], mmdt, kind="ExternalInput")
    cosK = nc.dram_tensor("cosK", [128, N], mmdt, kind="ExternalInput")
    sinK = nc.dram_tensor("sinK", [128, N], mmdt, kind="ExternalInput")
    outT = nc.dram_tensor("outT", [D, NQ], mmdt, kind="ExternalOutput")

    NKC = N // 128       # 16 key chunks
    NFC = D // 128       # 8 feature chunks
    NVC = D // 128       # 8 contraction chunks for V

    with ExitStack() as ctx:
        tc = ctx.enter_context(tile.TileContext(nc))

        const = ctx.enter_context(tc.tile_pool(name="const", bufs=1))

        # persistent sbuf tensors
        v_sb = const.tile([128, NKC, H * 65], mmdt)   # V' (keys, per-head 64+ones)
        q_sb = const.tile([128, NFC, NQ], mmdt)       # Q_rot.T
        k_sb = const.tile([128, NFC, N], mmdt)        # K_rot.T
        y_sb = const.tile([128, NFC, NQ], mmdt)       # Y.T (normalized attn out)
        bq_sb = const.tile([128, NFC], f32)
        bk_sb = const.tile([128, NFC], f32)
        bo_sb = const.tile([128, NFC], f32)
        ck_sb = const.tile([128, N], mmdt)
        sk_sb = const.tile([128, N], mmdt)
        cq_sb = const.tile([128, NQ], mmdt)
        sq_sb = const.tile([128, NQ], mmdt)
        wvb_sb = const.tile([128, H * 65], mmdt)
        xq_sb = const.tile([128, NFC // 2, 2, NQ], f8)

        _dmaq = [nc.sync, nc.scalar, nc.gpsimd]
        _dmaqi = [0]

        def dma_rr(dst, src_ap):
            eng = _dmaq[_dmaqi[0] % len(_dmaq)]
            _dmaqi[0] += 1
            eng.dma_start(dst, src_ap)

        def load_chunked(dst_tile, src_t, nchunks, splits=4):
            per = nchunks // splits if nchunks % splits == 0 else 1
            if per == 0:
                per = 1
            c = 0
            while c < nchunks:
                n = min(per, nchunks - c)
                dma_rr(
                    dst_tile[:, c:c + n, :],
                    src_t[c * 128:(c + n) * 128, :].rearrange(
                        "(c p) r -> p c r", p=128),
                )
                c += n

        def load_consts():
            nc.sync.dma_start(bq_sb[:], bq_d.rearrange("(c p) -> p c", p=128))
            nc.sync.dma_start(bk_sb[:], bk_d.rearrange("(c p) -> p c", p=128))
            nc.sync.dma_start(bo_sb[:], bo_d.rearrange("(c p) -> p c", p=128))
            nc.sync.dma_start(wvb_sb[:], wvb[:])
            nc.gpsimd.dma_start(ck_sb[:], cosK[:])
            nc.gpsimd.dma_start(sk_sb[:], sinK[:])
            nc.scalar.dma_start(cq_sb[:], cosQ[:])
            nc.scalar.dma_start(sq_sb[:], sinQ[:])

        rope_pool = ctx.enter_context(tc.tile_pool(name="rope", bufs=3))

        def load_pairs(dst_tile, src_t, rows):
            for c in range(NFC // 2):
                dma_rr(
                    dst_tile[:, c, :, :],
                    src_t[c * 128:(c + 1) * 128, :, :],
                )

        # Q/K weight column staging (per output-feature chunk, 2 in flight)
        wqp = ctx.enter_context(tc.tile_pool(name="wqstage", bufs=2))
        wkp = ctx.enter_context(tc.tile_pool(name="wkstage", bufs=2))
        _staged_wq = {}
        _staged_wk = {}

        def stage_wq(fc):
            wq = wqp.tile([128, NFC // 2, 2, 128], f8, tag="wq",
                          name=f"wq{fc}")
            dma_rr(wq[:, :, :, :], wq8T[:, fc, :])
            _staged_wq[fc] = wq
            return wq

        def stage_wk(fc):
            wk = wkp.tile([128, NFC // 2, 2, 128], f8, tag="wk",
                          name=f"wk{fc}")
            dma_rr(wk[:, :, :, :], wk8T[:, fc, :])
            _staged_wk[fc] = wk
            return wk

        def get_staged_wq(fc):
            return _staged_wq.pop(fc) if fc in _staged_wq else stage_wq(fc)

        def get_staged_wk(fc):
            return _staged_wk.pop(fc) if fc in _staged_wk else stage_wk(fc)

        # Persistent zeroed sin-term temporaries: pass rows stay zero forever;
        # only the 4x16 rotary rows are rewritten each block.
        tmpS_tiles = [
            const.tile([128, 1024], mmdt, tag=f"tmpS{i}", name=f"tmpS{i}")
            for i in (0, 1)
        ]
        for t in tmpS_tiles:
            nc.vector.memset(t[:], 0.0)
        _blk = [0]

        # Per-head feature layout (after the host permutation):
        #   [0:16) evens, [16:32) pass, [32:48) odds, [48:64) pass
        # so rotary partners are at +-32 partitions (quadrant aligned).
        # pc already carries the bias (folded into the psum eviction), so
        # every rope op is a plain bf16 tensor_tensor (2x DVE mode).
        def rope_1024(pc, cos_sb, sin_sb, dst_ap, r0):
            blk = _blk[0]
            _blk[0] += 1
            tmpC = rope_pool.tile([128, 1024], mmdt, tag="tmpC",
                                  name=f"tmpC{blk}")
            tmpS = tmpS_tiles[blk % 2]
            nc.vector.tensor_mul(tmpC[:], pc[:], cos_sb[:, r0:r0 + 1024])
            # sin part: out rows R read pc at the partner rows P = R +- 32;
            # the sin table is negated so the value at the partner location
            # is the sign-correct coefficient for row R.
            for h2 in (0, 64):
                nc.vector.tensor_mul(
                    tmpS[h2:h2 + 16, :],
                    pc[h2 + 32:h2 + 48, :],
                    sin_sb[h2 + 32:h2 + 48, r0:r0 + 1024],
                )
                nc.vector.tensor_mul(
                    tmpS[h2 + 32:h2 + 48, :],
                    pc[h2:h2 + 16, :],
                    sin_sb[h2:h2 + 16, r0:r0 + 1024],
                )
            nc.vector.tensor_add(dst_ap, tmpC[:], tmpS[:])

        def phase_v():
            with tc.tile_pool(name="vphase", bufs=1) as vp, \
                 tc.tile_pool(name="vpsum", bufs=8, space="PSUM") as vps:
                xv_sb = vp.tile([128, NVC, N], mmdt)
                wv_sb = vp.tile([128, NVC, H * 65], mmdt)
                for dc in range(NVC):
                    dma_rr(xv_sb[:, dc, :],
                           xvT[dc * 128:(dc + 1) * 128, :])
                    dma_rr(wv_sb[:, dc, :],
                           wvT[dc * 128:(dc + 1) * 128, :])
                load_consts()
                # prefetch next phase's input + first weight stages
                load_pairs(xq_sb, xq8T, NQ)
                stage_wq(0)
                stage_wk(0)
                for kc in range(NKC):
                    # the bufs=8 ring gives consecutive kc disjoint bank
                    # quads: per-bank accumulate WAW distance is 8 matmuls
                    # (~870ns), clear of the psum-accumulate stall.
                    pss = [vps.tile([128, 260], f32, tag="vps",
                                    name=f"vps{kc}_{i}")
                           for i in range(4)]
                    for dc in range(NVC):
                        for nf in range(4):
                            nc.tensor.matmul(
                                pss[nf][:],
                                lhsT=xv_sb[:, dc, kc * 128:(kc + 1) * 128],
                                rhs=wv_sb[:, dc, nf * 260:(nf + 1) * 260],
                                start=(dc == 0),
                                stop=(dc == NVC - 1),
                            )
                    # bias + ones column folded into the DVE eviction add
                    # against the host-broadcast wvb tile.
                    for nf in range(4):
                        nc.vector.tensor_add(
                            v_sb[:, kc, nf * 260:(nf + 1) * 260], pss[nf][:],
                            wvb_sb[:, nf * 260:(nf + 1) * 260],
                        )

        def make_proj_pieces(fc, w_sb, x_sb, cos_sb, sin_sb, b_sb, dst,
                             ncols, qk_psum):
            """Q or K projection for one feature chunk as a list of small
            closures (mostly one matmul each) to be spread across the kc
            iterations of the attention loop: same-bank accumulating
            matmuls land ~1us apart (no psum-accumulate stall) and the PE
            never waits long on an eviction."""
            pieces = []
            for pairi in range(ncols // 1024):
                pc = [None]

                def start_block(pairi=pairi, pc=pc):
                    pc[0] = rope_pool.tile([128, 1024], mmdt, tag="pscopy",
                                           name=f"pc{fc}_{pairi}")

                pieces.append(start_block)
                for sub in range(2):
                    half = pairi * 2 + sub
                    ps = [None]

                    def start_half(ps=ps, half=half):
                        ps[0] = qk_psum.tile([128, 512], f32, tag="qkps",
                                             name=f"qkps{fc}_{half}")

                    pieces.append(start_half)
                    for dcp in range(NFC // 2):
                        def mm(dcp=dcp, half=half, ps=ps):
                            nc.tensor.matmul(
                                ps[0][:],
                                lhsT=w_sb[:, dcp, :, :],
                                rhs=x_sb[:, dcp, :,
                                         half * 512:(half + 1) * 512],
                                start=(dcp == 0),
                                stop=(dcp == NFC // 2 - 1),
                                perf_mode=mybir.MatmulPerfMode.DoubleRow,
                            )
                        pieces.append(mm)

                    def evict(sub=sub, ps=ps, pc=pc):
                        # on ACT: the next half's start=True matmul WAW-waits
                        # this eviction; ACT's lag is bounded (~1 exp), while
                        # the DVE backlog would stall the in-order PE queue.
                        nc.scalar.activation(
                            pc[0][:, sub * 512:(sub + 1) * 512], ps[0][:],
                            AF.Identity, bias=b_sb[:, fc:fc + 1])
                    pieces.append(evict)

                def rope_fin(pairi=pairi, pc=pc):
                    r0 = pairi * 1024
                    rope_1024(pc[0], cos_sb, sin_sb,
                              dst[:, fc, r0:r0 + 1024], r0)
                pieces.append(rope_fin)
            return pieces

        def make_qk_pieces(fc, xk_sb, qk_psum):
            wq = get_staged_wq(fc)
            wk = get_staged_wk(fc)
            return (
                make_proj_pieces(fc, wq, xq_sb, cq_sb, sq_sb, bq_sb, q_sb,
                                 NQ, qk_psum)
                + make_proj_pieces(fc, wk, xk_sb, ck_sb, sk_sb, bk_sb, k_sb,
                                   N, qk_psum)
            )

        def attn_pair(fc, spool, poApool, poBpool, po1pool, ppool, npool,
                      pieces):
            """Attention for heads (2*fc, 2*fc+1), two query-half passes.
            h0 accumulates into 2 rotating psum banks (merged on DVE);
            h1 single-banked (ACT-paced WAW spacing is sufficient).
            pieces (next pair's K projection) spread into the kc loop."""
            h0, h1 = 2 * fc, 2 * fc + 1

            for qn in range(2):
                poA = poApool.tile([65, 512], f32, tag="poA",
                                   name=f"poA{fc}_{qn}")
                poB = poBpool.tile([65, 512], f32, tag="poB",
                                   name=f"poB{fc}_{qn}")
                po1 = po1pool.tile([65, 512], f32, tag="po1",
                                   name=f"po1{fc}_{qn}")
                pts = {}

                def scores_kc(kc):
                    st = spool.tile([128, 2, 512], f32, tag="st",
                                    name=f"st{fc}_{qn}_{kc}")
                    for i, hb in enumerate((0, 64)):
                        nc.tensor.matmul(
                            st[:, i, :],
                            lhsT=k_sb[hb:hb + 64, fc, kc * 128:(kc + 1) * 128],
                            rhs=q_sb[hb:hb + 64, fc,
                                     qn * 512:(qn + 1) * 512],
                            start=True,
                            stop=True,
                        )
                    return st

                def exp_kc(kc, st):
                    pt = ppool.tile([128, 2, 512], mmdt, tag="pt",
                                    name=f"pt{fc}_{qn}_{kc}")
                    if SKIPEXP:
                        if kc == 0:
                            nc.vector.memset(pt[:], 0.01)
                            pts["static"] = pt
                        pt = pts["static"]
                    else:
                        nc.scalar.activation(pt[:], st[:], AF.Exp,
                                             scale=1.0 / (8.0 * 1024.0))
                    pts[kc] = pt

                def attnv_kc(kc):
                    # h0 rotates over 2 accumulator banks (merged on DVE at
                    # normalize); h1 single-banked. Same-bank accumulating
                    # matmuls closer than ~850ns stall the PE.
                    dst0 = poA if kc % 2 == 0 else poB
                    nc.tensor.matmul(
                        dst0[:],
                        lhsT=v_sb[:, kc, h0 * 65:(h0 + 1) * 65],
                        rhs=pts[kc][:, 0, :],
                        start=(kc < 2),
                        stop=(kc >= NKC - 2),
                    )
                    nc.tensor.matmul(
                        po1[:],
                        lhsT=v_sb[:, kc, h1 * 65:(h1 + 1) * 65],
                        rhs=pts[kc][:, 1, :],
                        start=(kc == 0),
                        stop=(kc == NKC - 1),
                    )
                    del pts[kc]

                # software pipeline: scores(kc) || exp(kc) || attnV(kc-1),
                # K-projection pieces spread between iterations (front-
                # loaded so the DVE is free near the group boundary)
                st = scores_kc(0)
                exp_kc(0, st)
                for kc in range(1, NKC):
                    st = scores_kc(kc)
                    attnv_kc(kc - 1)
                    exp_kc(kc, st)
                    # 42 pieces spread over the 30 kc-slots of the 2 qn
                    # passes, front-loaded within each pass
                    npop = 0 if SKIPPROJ else (2 if kc <= 11 else 1)
                    for _ in range(npop):
                        if pieces:
                            pieces.pop(0)()
                attnv_kc(NKC - 1)

                # psum-freeing copies on ACT (bounded lag): the next group's
                # attnV start=True matmuls WAW-wait these reads.
                ya = npool.tile([65, 512], f32, tag="ya",
                                name=f"ya{fc}_{qn}")
                nc.scalar.activation(ya[:], po1[:], AF.Identity)
                yb = npool.tile([65, 512], f32, tag="yb",
                                name=f"yb{fc}_{qn}")
                nc.scalar.activation(yb[:], poA[:], AF.Identity)
                ym = npool.tile([65, 512], f32, tag="ym",
                                name=f"ym{fc}_{qn}")
                nc.vector.tensor_add(ym[:], yb[:], poB[:])
                for i, (yy, hb) in enumerate(((ym, 0), (ya, 64))):
                    rz = npool.tile([1, 512], f32, tag=f"rz{i}", bufs=1,
                                    name=f"rz{i}_{fc}_{qn}")
                    nc.vector.reciprocal(rz[:], yy[64:65, :])
                    rzb = npool.tile([64, 512], f32, tag=f"rzb{i}", bufs=1,
                                     name=f"rzb{i}_{fc}_{qn}")
                    nc.gpsimd.partition_broadcast(rzb[:], rz[:])
                    nc.vector.tensor_mul(
                        y_sb[hb:hb + 64, fc, qn * 512:(qn + 1) * 512],
                        yy[0:64, :], rzb[:]
                    )

        def phase_qk_attn(do_attn):
            with tc.tile_pool(name="kstage", bufs=1) as kp:
                xk_sb = kp.tile([128, NFC // 2, 2, N], f8)
                wo_sb = kp.tile([128, NFC, D], mmdt)
                load_pairs(xk_sb, xk8T, N)
                # prefetch output-projection weights into the attn window
                load_chunked(wo_sb, woT, NFC)
                phase_qk_attn_inner(do_attn, xk_sb)
                # attn psum pools are closed; out projection runs here so
                # wo_sb (kstage) stays alive
                if PHASES >= 5:
                    phase_out(wo_sb)

        def phase_qk_attn_inner(do_attn, xk_sb):
            with tc.tile_pool(name="qkpsum", bufs=1, space="PSUM") as qk_psum:
                if do_attn:
                    with tc.tile_pool(name="spsum", bufs=2,
                                      space="PSUM") as spool, \
                         tc.tile_pool(name="poApsum", bufs=1,
                                      space="PSUM") as poApool, \
                         tc.tile_pool(name="poBpsum", bufs=1,
                                      space="PSUM") as poBpool, \
                         tc.tile_pool(name="po1psum", bufs=1,
                                      space="PSUM") as po1pool, \
                         tc.tile_pool(name="ptpool", bufs=5) as ppool, \
                         tc.tile_pool(name="npool", bufs=2) as npool:
                        # pair 0's Q+K projections run dense; pair p's attn
                        # spreads pair p+1's projection pieces.
                        for piece in make_qk_pieces(0, xk_sb, qk_psum):
                            piece()
                        for fc in range(NFC):
                            if fc + 1 < NFC:
                                pieces = make_qk_pieces(fc + 1, xk_sb,
                                                        qk_psum)
                            else:
                                pieces = []
                            attn_pair(fc, spool, poApool, poBpool, po1pool,
                                      ppool, npool, pieces)
                            assert SKIPPROJ or not pieces, \
                                f"{len(pieces)} pieces left"
                else:
                    for fc in range(NFC):
                        for piece in make_qk_pieces(fc, xk_sb, qk_psum):
                            piece()
                    nc.vector.memset(y_sb[:], 0.0)

        def phase_out(wo_sb):
            # 4 dmc x 2 rn rotating accumulator banks: per-bank WAW
            # distance is 8 matmuls, no accumulate stall.
            with tc.tile_pool(name="owork", bufs=3) as owork, \
                 tc.tile_pool(name="opsum2", bufs=8, space="PSUM") as ops2:
                for quad in range(2):
                    pss = {}
                    for d4 in range(4):
                        for rn in range(2):
                            pss[(d4, rn)] = ops2.tile(
                                [128, 512], f32, tag="ops",
                                name=f"ops{quad}_{d4}_{rn}")
                    for fc in range(NFC):
                        for d4 in range(4):
                            dmc = quad * 4 + d4
                            for rn in range(2):
                                nc.tensor.matmul(
                                    pss[(d4, rn)][:],
                                    lhsT=wo_sb[:, fc,
                                               dmc * 128:(dmc + 1) * 128],
                                    rhs=y_sb[:, fc, rn * 512:(rn + 1) * 512],
                                    start=(fc == 0),
                                    stop=(fc == NFC - 1),
                                )
                    for d4 in range(4):
                        dmc = quad * 4 + d4
                        ob = owork.tile([128, NQ], mmdt, tag="ob",
                                        name=f"ob{quad}_{d4}")
                        for rn in range(2):
                            nc.vector.tensor_scalar_add(
                                ob[:, rn * 512:(rn + 1) * 512],
                                pss[(d4, rn)][:],
                                bo_sb[:, dmc:dmc + 1])
                        eng = nc.sync if d4 % 2 == 0 else nc.gpsimd
                        eng.dma_start(outT[dmc * 128:(dmc + 1) * 128, :],
                                      ob[:])

        def all_phases():
            if PHASES >= 1:
                phase_v()
            if PHASES >= 2:
                phase_qk_attn(do_attn=(PHASES >= 4))
            else:
                nc.vector.memset(y_sb[:], 0.0)
            if PHASES < 5:
                with tc.tile_pool(name="dummy", bufs=1) as dp:
                    zb = dp.tile([128, NQ], mmdt)
                    nc.vector.memset(zb[:], 0.0)
                    for dmc in range(NFC):
                        nc.sync.dma_start(outT[dmc * 128:(dmc + 1) * 128, :],
                                          zb[:])

        if KLOOP > 1:
            with tc.For_i(0, KLOOP, 1,
                          hint_engines=(mybir.EngineType.PE,
                                        mybir.EngineType.DVE,
                                        mybir.EngineType.Activation),
                          staggered_reset=True):
                all_phases()
        else:
            all_phases()

    nc.compile()
    return nc


def _rope_tables(positions):
    """cos/sin tables [128, len(positions)] for the permuted transposed
    layout: partition p (within a 2-head feature chunk), j = p % 64:
    j<16: freq j (cos, -sin); 32<=j<48: freq j-32 (cos, +sin); else (1, 0)."""
    inv_freq = 1.0 / (THETA ** (np.arange(0, ROT, 2, dtype=np.float64) / ROT))  # [16]
    t = np.asarray(positions, dtype=np.float64)
    ang = t[None, :] * inv_freq[:, None]  # [16, nt]
    c, s = np.cos(ang), np.sin(ang)
    cos_tab = np.ones((128, len(positions)), dtype=np.float64)
    sin_tab = np.zeros((128, len(positions)), dtype=np.float64)
    for h2 in (0, 64):
        cos_tab[h2:h2 + 16] = c
        cos_tab[h2 + 32:h2 + 48] = c
        # NEGATED sin coefficients (+s for evens, -s for odds): the device
        # reads the sin operand at the +-32 partner partition, where the
        # negated opposite-sign value equals the row's own coefficient.
        sin_tab[h2:h2 + 16] = s
        sin_tab[h2 + 32:h2 + 48] = -s
    return cos_tab.astype(np.float32), sin_tab.astype(np.float32)


def _head_perm():
    """Feature permutation applied to rows of Wq/Wk (and bq/bk): within each
    head's 64 outputs -> [evens(16), pass 32:48, odds(16), pass 48:64]."""
    out = np.empty(D, dtype=np.int64)
    for h in range(H):
        base = h * DK
        out[base:base + HALF] = base + np.arange(0, ROT, 2)
        out[base + HALF:base + ROT] = base + np.arange(ROT, ROT + HALF)
        out[base + ROT:base + ROT + HALF] = base + np.arange(1, ROT, 2)
        out[base + ROT + HALF:base + DK] = base + np.arange(ROT + HALF, DK)
    return out


def _prep_inputs(query, key, value, Wq, bq, Wk, bk, Wv, bv, Wo, bo,
                 mm_dtype_name="bfloat16"):
    import ml_dtypes

    np_mm = ml_dtypes.bfloat16 if mm_dtype_name == "bfloat16" else np.float32

    query = np.asarray(query, np.float32)
    key = np.asarray(key, np.float32)
    value = np.asarray(value, np.float32)
    Wq, bq = np.asarray(Wq, np.float32), np.asarray(bq, np.float32)
    Wk, bk = np.asarray(Wk, np.float32), np.asarray(bk, np.float32)
    Wv, bv = np.asarray(Wv, np.float32), np.asarray(bv, np.float32)
    Wo, bo = np.asarray(Wo, np.float32), np.asarray(bo, np.float32)

    perm = _head_perm()
    Wq_p, bq_p = Wq[perm], bq[perm]
    Wk_p, bk_p = Wk[perm], bk[perm]
    np_f8 = ml_dtypes.float8_e4m3

    def _pairs(a):  # [D, cols] -> [D//2, 2, cols] DoubleRow interleave
        d, cols = a.shape
        out = np.empty((d // 2, 2, cols), a.dtype)
        for dcp in range(d // 256):
            for j in (0, 1):
                out[dcp * 128:(dcp + 1) * 128, j, :] = \
                    a[(2 * dcp + j) * 128:(2 * dcp + j + 1) * 128, :]
        return out

    def _wflat(wt):  # [D, D] lhsT -> [128, NFC, 1024] = [p, fc, (c j m)]
        t = wt.reshape(4, 2, 128, H // 2, 128)  # [c, j, p, fc, m]
        return np.ascontiguousarray(
            t.transpose(2, 3, 0, 1, 4).reshape(128, H // 2, 1024))

    # Q/K path in fp8: weights prescaled by 32 (folded out in the exp scale)
    wq8T = _wflat((Wq_p.T * 32.0).astype(np_f8))
    wk8T = _wflat((Wk_p.T * 32.0).astype(np_f8))
    bq_p = bq_p * 32.0
    bk_p = bk_p * 32.0
    woT = np.ascontiguousarray(Wo.T).astype(np_mm)

    # W_v' : [D, H*65] plus a broadcast bias/ones tile wvb [128, H*65]
    wvT = np.zeros((D, H * 65), np.float32)
    wvb = np.zeros((1, H * 65), np.float32)
    for h in range(H):
        cols = slice(h * 65, h * 65 + 64)
        wvT[:D, cols] = Wv[h * DK:(h + 1) * DK, :].T
        wvb[0, cols] = bv[h * DK:(h + 1) * DK]
        wvb[0, h * 65 + 64] = 1.0
    wvT = wvT.astype(np_mm)
    wvb_bc = np.ascontiguousarray(np.repeat(wvb, 128, axis=0)).astype(np_mm)

    cos_all, sin_all = _rope_tables(np.arange(N))

    in_maps = []
    for core in range(NCORES):
        b, qh = core // 2, core % 2
        rows = slice(qh * NQ, (qh + 1) * NQ)
        xq8T = np.ascontiguousarray(
            _pairs(query[b, rows, :].T.astype(np_f8)))
        xk8T = np.ascontiguousarray(_pairs(key[b].T.astype(np_f8)))
        xvT = np.ascontiguousarray(value[b].T).astype(np_mm)
        in_maps.append({
            "xq8T": xq8T,
            "xk8T": xk8T,
            "xvT": xvT,
            "wq8T": wq8T, "wk8T": wk8T, "wvT": wvT, "woT": woT,
            "wvb": wvb_bc,
            "bq_d": bq_p, "bk_d": bk_p, "bo_d": bo,
            "cosQ": np.ascontiguousarray(cos_all[:, rows]).astype(np_mm),
            "sinQ": np.ascontiguousarray(sin_all[:, rows]).astype(np_mm),
            "cosK": cos_all.astype(np_mm),
            "sinK": sin_all.astype(np_mm),
        })
    return in_maps


def kernel(query, key, value, Wq, bq, Wk, bk, Wv, bv, Wo, bo):
    from concourse import bass_utils

    mm_dtype_name = "bfloat16"
    if mm_dtype_name not in _PROGRAM_CACHE:
        _PROGRAM_CACHE[mm_dtype_name] = _build_program(mm_dtype_name)
    nc = _PROGRAM_CACHE[mm_dtype_name]

    in_maps = _prep_inputs(query, key, value, Wq, bq, Wk, bk, Wv, bv, Wo, bo,
                           mm_dtype_name)

    res = bass_utils.run_bass_kernel_spmd(
        nc, in_maps, core_ids=list(range(NCORES))
    )

    out = np.empty((B, N, D), np.float32)
    for core in range(NCORES):
        b, qh = core // 2, core % 2
        out[b, qh * NQ:(qh + 1) * NQ, :] = \
            res.results[core]["outT"].T.astype(np.float32)
    return out


# revision 42
# speedup vs baseline: 1.1767x; 1.0456x over previous
"""Trainium2 Bass kernel for MultiHeadedAttention with RoPE (v3).

Problem: b=4, n=2048, d=1024, H=16 heads, dk=64, rotary on first 32 dims
(interleaved pairs, theta=10000, lucidrains convention).

Sharding: 8 cores = 4 batches x 2 query-halves (data parallel). Each core
computes the full K/V projections for its batch (replicated across the 2
query-half siblings) and attention + output projection for its 1024 query
rows. No collectives needed; host gathers/concatenates.

Device-side layout strategy (all "transposed", features on partitions):
  - Host passes X.T (d-major) so projections need no on-device transposes.
  - RoPE: host permutes Wq/Wk output features per head to [evens(16),
    pass(16), odds(16), pass(16)] so the interleaved pair rotation becomes
    a +-32 partition-offset multiply against cos/sin tables.
  - scores.T = K_rot.T' @ Q_rot.T per head; the two heads of a
    128-partition chunk run as concurrent row-group matmuls (row tiling).
  - softmax: exp without max-subtraction (scores O(1)); normalizer Z from
    a ones-column appended to V; 1/Z broadcast across partitions via
    gpsimd partition_broadcast.

v3 structural changes vs v2 (all driven by HW microbenchmarks — the
CoreSim cost model does not model LDWEIGHTS cost or PSUM-accumulate
stalls, which together explained most of the HW-vs-sim gap):
  - attnV psum accumulation rotated across banks: same-bank accumulating
    matmuls closer than ~850ns stall the PE (and trip HAM re-throttle).
    h0 rotates over 2 banks (merged at normalize), h1 stays single-banked
    (its WAW distance is ACT-paced, which is enough).
  - Q and K projections no longer emitted as contiguous 4-matmul
    accumulation chains (WAW-1): split into single-matmul pieces spread
    ~1-2 per kc iteration of the attention loop, so each chain's matmuls
    land >1us apart and fill PE time under the ACT exp pacing.
  - all psum evictions moved to ACT (activation Identity with the bias as
    the per-partition bias operand): ACT's queue lag is bounded by ~1 exp,
    while a DVE backlog stalls the strictly in-order PE queue at the next
    WAW-dependent matmul.
  - rope computed with plain bf16 tensor_tensor muls (2x DVE mode) after
    folding the bias into the ACT psum eviction; the v2
    scalar_tensor_tensor ops ran at 1x.
  - V projection accumulates over an 8-bank ring (2 kc in flight).
  - V' bias+ones folded into a DVE eviction add against a host-broadcast
    bias tile (kills the K=1 bias matmuls and keeps ACT exp-only).
  - out projection: 8 rotating psum banks (4 dmc x 2 rn), WAW distance 8.
"""

import os

import numpy as np

B, N, D = 4, 2048, 1024
H, DK = 16, 64
ROT, HALF = 32, 16
THETA = 10000.0
NCORES = 8
NQ = N // 2  # query rows per core

_PROGRAM_CACHE = {}


def _build_program(mm_dtype_name="bfloat16"):
    import concourse.tile as tile
    from concourse import bacc, mybir
    from contextlib import ExitStack

    PHASES = int(os.environ.get("KPHASES", "9"))  # debug bisect knob
    KLOOP = int(os.environ.get("KLOOP", "1"))      # hw-loop repeat (timing)
    SKIPEXP = os.environ.get("KSKIPEXP", "0") == "1"    # timing-only bisect
    SKIPPROJ = os.environ.get("KSKIPPROJ", "0") == "1"  # timing-only bisect

    f32 = mybir.dt.float32
    mmdt = getattr(mybir.dt, mm_dtype_name)
    AF = mybir.ActivationFunctionType

    NFC_ = D // 128
    nc = bacc.Bacc("TRN2", target_bir_lowering=False)

    # DRAM I/O (per core). All *T tensors are feature-major (transposed).
    f8 = mybir.dt.float8e4
    xq8T = nc.dram_tensor("xq8T", [D // 2, 2, NQ], f8, kind="ExternalInput")
    xk8T = nc.dram_tensor("xk8T", [D // 2, 2, NQ], f8, kind="ExternalInput")
    xvT = nc.dram_tensor("xvT", [D, NQ], mmdt, kind="ExternalInput")
    wq8T = nc.dram_tensor("wq8T", [128, NFC_, 1024], f8, kind="ExternalInput")
    wk8T = nc.dram_tensor("wk8T", [128, NFC_, 1024], f8, kind="ExternalInput")
    wvT = nc.dram_tensor("wvT", [D, H * 65], mmdt, kind="ExternalInput")
    wvb = nc.dram_tensor("wvb", [128, H * 65], mmdt, kind="ExternalInput")
    woT = nc.dram_tensor("woT", [D, D], mmdt, kind="ExternalInput")
    bq_d = nc.dram_tensor("bq_d", [D], f32, kind="ExternalInput")
    bk_d = nc.dram_tensor("bk_d", [D], f32, kind="ExternalInput")
    bo_d = nc.dram_tensor("bo_d", [D], f32, kind="ExternalInput")
    cosQ = nc.dram_tensor("cosQ", [128, NQ], mmdt, kind="ExternalInput")
    sinQ = nc.dram_tensor("sinQ", [128, NQ], mmdt, kind="ExternalInput")
    outT = nc.dram_tensor("outT", [D, NQ], mmdt, kind="ExternalOutput")

    NKC = N // 128       # 16 key chunks
    NFC = D // 128       # 8 feature chunks
    NVC = D // 128       # 8 contraction chunks for V

    with ExitStack() as ctx:
        tc = ctx.enter_context(tile.TileContext(nc))

        const = ctx.enter_context(tc.tile_pool(name="const", bufs=1))

        # persistent sbuf tensors
        v_sb = const.tile([128, NKC, H * 65], mmdt)   # V' (keys, per-head 64+ones)
        q_sb = const.tile([128, NFC, NQ], mmdt)       # Q_rot.T
        k_sb = const.tile([128, NFC, N], mmdt)        # K_rot.T
        y_sb = const.tile([128, NFC, NQ], mmdt)       # Y.T (normalized attn out)
        bq_sb = const.tile([128, NFC], f32)
        bk_sb = const.tile([128, NFC], f32)
        bo_sb = const.tile([128, NFC], f32)
        cq_sb = const.tile([128, NQ], mmdt)
        sq_sb = const.tile([128, NQ], mmdt)
        wvb_sb = const.tile([128, H * 65], mmdt)
        xq_sb = const.tile([128, NFC // 2, 2, NQ], f8)

        _dmaq = [nc.sync, nc.scalar, nc.gpsimd]
        _dmaqi = [0]

        def dma_rr(dst, src_ap):
            eng = _dmaq[_dmaqi[0] % len(_dmaq)]
            _dmaqi[0] += 1
            eng.dma_start(dst, src_ap)

        def load_chunked(dst_tile, src_t, nchunks, splits=4):
            per = nchunks // splits if nchunks % splits == 0 else 1
            if per == 0:
                per = 1
            c = 0
            while c < nchunks:
                n = min(per, nchunks - c)
                dma_rr(
                    dst_tile[:, c:c + n, :],
                    src_t[c * 128:(c + n) * 128, :].rearrange(
                        "(c p) r -> p c r", p=128),
                )
                c += n

        def load_consts():
            nc.sync.dma_start(bq_sb[:], bq_d.rearrange("(c p) -> p c", p=128))
            nc.sync.dma_start(bk_sb[:], bk_d.rearrange("(c p) -> p c", p=128))
            nc.sync.dma_start(bo_sb[:], bo_d.rearrange("(c p) -> p c", p=128))
            nc.sync.dma_start(wvb_sb[:], wvb[:])
            nc.scalar.dma_start(cq_sb[:], cosQ[:])
            nc.scalar.dma_start(sq_sb[:], sinQ[:])

        rope_pool = ctx.enter_context(tc.tile_pool(name="rope", bufs=3))

        def load_pairs(dst_tile, src_t, rows):
            for c in range(NFC // 2):
                dma_rr(
                    dst_tile[:, c, :, :],
                    src_t[c * 128:(c + 1) * 128, :, :],
                )

        # Q/K weight column staging (per output-feature chunk, 2 in flight)
        wqp = ctx.enter_context(tc.tile_pool(name="wqstage", bufs=2))
        wkp = ctx.enter_context(tc.tile_pool(name="wkstage", bufs=2))
        _staged_wq = {}
        _staged_wk = {}

        def stage_wq(fc):
            wq = wqp.tile([128, NFC // 2, 2, 128], f8, tag="wq",
                          name=f"wq{fc}")
            dma_rr(wq[:, :, :, :], wq8T[:, fc, :])
            _staged_wq[fc] = wq
            return wq

        def stage_wk(fc):
            wk = wkp.tile([128, NFC // 2, 2, 128], f8, tag="wk",
                          name=f"wk{fc}")
            dma_rr(wk[:, :, :, :], wk8T[:, fc, :])
            _staged_wk[fc] = wk
            return wk

        def get_staged_wq(fc):
            return _staged_wq.pop(fc) if fc in _staged_wq else stage_wq(fc)

        def get_staged_wk(fc):
            return _staged_wk.pop(fc) if fc in _staged_wk else stage_wk(fc)

        # Persistent zeroed sin-term temporaries: pass rows stay zero forever;
        # only the 4x16 rotary rows are rewritten each block.
        tmpS_tiles = [
            const.tile([128, 1024], mmdt, tag=f"tmpS{i}", name=f"tmpS{i}")
            for i in (0, 1)
        ]
        for t in tmpS_tiles:
            nc.vector.memset(t[:], 0.0)
        _blk = [0]

        # Per-head feature layout (after the host permutation):
        #   [0:16) evens, [16:32) pass, [32:48) odds, [48:64) pass
        # so rotary partners are at +-32 partitions (quadrant aligned).
        # pc already carries the bias (folded into the psum eviction), so
        # every rope op is a plain bf16 tensor_tensor (2x DVE mode).
        def rope_1024(pc, cos_sb, sin_sb, dst_ap, r0):
            blk = _blk[0]
            _blk[0] += 1
            tmpC = rope_pool.tile([128, 1024], mmdt, tag="tmpC",
                                  name=f"tmpC{blk}")
            tmpS = tmpS_tiles[blk % 2]
            nc.vector.tensor_mul(tmpC[:], pc[:], cos_sb[:, r0:r0 + 1024])
            # sin part: out rows R read pc at the partner rows P = R +- 32;
            # the sin table is negated so the value at the partner location
            # is the sign-correct coefficient for row R.
            for h2 in (0, 64):
                nc.vector.tensor_mul(
                    tmpS[h2:h2 + 16, :],
                    pc[h2 + 32:h2 + 48, :],
                    sin_sb[h2 + 32:h2 + 48, r0:r0 + 1024],
                )
                nc.vector.tensor_mul(
                    tmpS[h2 + 32:h2 + 48, :],
                    pc[h2:h2 + 16, :],
                    sin_sb[h2:h2 + 16, r0:r0 + 1024],
                )
            nc.vector.tensor_add(dst_ap, tmpC[:], tmpS[:])

        def phase_v(k_in, v_in):
            """Half V projection (this core's 1024 keys) with this core's
            half K projection pieces interleaved between the dc-groups;
            results shipped to DRAM for the sibling AllGather."""
            with tc.tile_pool(name="vphase", bufs=1) as vp, \
                 tc.tile_pool(name="vqkpsum", bufs=1, space="PSUM") as qk_psum, \
                 tc.tile_pool(name="vpsum", bufs=7, space="PSUM") as vps:
                xv_sb = vp.tile([128, NVC, NQ], mmdt)
                wv_sb = vp.tile([128, NVC, H * 65], mmdt)
                xk_sb = vp.tile([128, NFC // 2, 2, NQ], f8)
                for dc in range(NVC):
                    dma_rr(xv_sb[:, dc, :],
                           xvT[dc * 128:(dc + 1) * 128, :])
                    dma_rr(wv_sb[:, dc, :],
                           wvT[dc * 128:(dc + 1) * 128, :])
                load_consts()
                # prefetch next phase's input + first weight stages
                load_pairs(xq_sb, xq8T, NQ)
                load_pairs(xk_sb, xk8T, NQ)
                stage_wq(0)
                # this core's K projection (keys == its q rows, so the rope
                # tables are shared with Q) as spread pieces
                kpieces = []
                for fc in range(NFC):
                    wkc = [None]

                    def kstage(fc=fc, wkc=wkc):
                        wkc[0] = get_staged_wk(fc)
                    kpieces.append(kstage)
                    kpieces.extend(make_proj_pieces(
                        fc, wkc, xk_sb, cq_sb, sq_sb, bk_sb, None,
                        NQ, qk_psum, ship_dram=k_in))
                for kc in range(NKC // 2):
                    # the bufs=8 ring gives consecutive kc disjoint bank
                    # quads: per-bank accumulate WAW distance is 8 matmuls
                    # (~870ns), clear of the psum-accumulate stall.
                    pss = [vps.tile([128, 260], f32, tag="vps",
                                    name=f"vps{kc}_{i}")
                           for i in range(4)]
                    for dc in range(NVC):
                        for nf in range(4):
                            nc.tensor.matmul(
                                pss[nf][:],
                                lhsT=xv_sb[:, dc, kc * 128:(kc + 1) * 128],
                                rhs=wv_sb[:, dc, nf * 260:(nf + 1) * 260],
                                start=(dc == 0),
                                stop=(dc == NVC - 1),
                            )
                        for _ in range(2):
                            if kpieces:
                                kpieces.pop(0)()
                    # bias + ones column folded into the DVE eviction add
                    # against the host-broadcast wvb tile, into a staging
                    # tile shipped to DRAM for the gather.
                    vship = rope_pool.tile([128, H * 65], mmdt, tag="vship",
                                           name=f"vship{kc}")
                    for nf in range(4):
                        nc.vector.tensor_add(
                            vship[:, nf * 260:(nf + 1) * 260], pss[nf][:],
                            wvb_sb[:, nf * 260:(nf + 1) * 260],
                        )
                    dma_rr(v_in[kc], vship[:])
                while kpieces:
                    kpieces.pop(0)()

        def make_proj_pieces(fc, w_cell, x_sb, cos_sb, sin_sb, b_sb, dst,
                             ncols, qk_psum, ship_dram=None):
            """Q or K projection for one feature chunk as a list of small
            closures (mostly one matmul each) to be spread across the kc
            iterations of the attention loop: same-bank accumulating
            matmuls land ~1us apart (no psum-accumulate stall) and the PE
            never waits long on an eviction."""
            pieces = []
            for pairi in range(ncols // 1024):
                pc = [None]

                def start_block(pairi=pairi, pc=pc):
                    pc[0] = rope_pool.tile([128, 1024], mmdt, tag="pscopy",
                                           name=f"pc{fc}_{pairi}")

                pieces.append(start_block)
                for sub in range(2):
                    half = pairi * 2 + sub
                    ps = [None]

                    def start_half(ps=ps, half=half):
                        ps[0] = qk_psum.tile([128, 512], f32, tag="qkps",
                                             name=f"qkps{fc}_{half}")

                    pieces.append(start_half)
                    for dcp in range(NFC // 2):
                        def mm(dcp=dcp, half=half, ps=ps):
                            nc.tensor.matmul(
                                ps[0][:],
                                lhsT=w_cell[0][:, dcp, :, :],
                                rhs=x_sb[:, dcp, :,
                                         half * 512:(half + 1) * 512],
                                start=(dcp == 0),
                                stop=(dcp == NFC // 2 - 1),
                                perf_mode=mybir.MatmulPerfMode.DoubleRow,
                            )
                        pieces.append(mm)

                    def evict(sub=sub, ps=ps, pc=pc):
                        # on ACT: the next half's start=True matmul WAW-waits
                        # this eviction; ACT's lag is bounded (~1 exp), while
                        # the DVE backlog would stall the in-order PE queue.
                        nc.scalar.activation(
                            pc[0][:, sub * 512:(sub + 1) * 512], ps[0][:],
                            AF.Identity, bias=b_sb[:, fc:fc + 1])
                    pieces.append(evict)

                if ship_dram is None:
                    def rope_fin(pairi=pairi, pc=pc):
                        r0 = pairi * 1024
                        rope_1024(pc[0], cos_sb, sin_sb,
                                  dst[:, fc, r0:r0 + 1024], r0)
                    pieces.append(rope_fin)
                else:
                    def rope_ship(pairi=pairi, pc=pc):
                        ktmp = rope_pool.tile([128, 1024], mmdt, tag="kship",
                                              name=f"kship{fc}")
                        rope_1024(pc[0], cos_sb, sin_sb, ktmp[:],
                                  pairi * 1024)
                        dma_rr(ship_dram[fc], ktmp[:])
                    pieces.append(rope_ship)
            return pieces

        def make_q_pieces(fc, qk_psum):
            wq = [get_staged_wq(fc)]
            return make_proj_pieces(fc, wq, xq_sb, cq_sb, sq_sb, bq_sb, q_sb,
                                    NQ, qk_psum)

        def attn_pair(fc, spool, poApool, poBpool, po1pool, ppool, npool,
                      pieces):
            """Attention for heads (2*fc, 2*fc+1), two query-half passes.
            h0 accumulates into 2 rotating psum banks (merged on DVE);
            h1 single-banked (ACT-paced WAW spacing is sufficient).
            pieces (next pair's K projection) spread into the kc loop."""
            h0, h1 = 2 * fc, 2 * fc + 1

            for qn in range(2):
                poA = poApool.tile([65, 512], f32, tag="poA",
                                   name=f"poA{fc}_{qn}")
                poB = poBpool.tile([65, 512], f32, tag="poB",
                                   name=f"poB{fc}_{qn}")
                po1 = po1pool.tile([65, 512], f32, tag="po1",
                                   name=f"po1{fc}_{qn}")
                pts = {}

                def scores_kc(kc):
                    st = spool.tile([128, 2, 512], f32, tag="st",
                                    name=f"st{fc}_{qn}_{kc}")
                    for i, hb in enumerate((0, 64)):
                        nc.tensor.matmul(
                            st[:, i, :],
                            lhsT=k_sb[hb:hb + 64, fc, kc * 128:(kc + 1) * 128],
                            rhs=q_sb[hb:hb + 64, fc,
                                     qn * 512:(qn + 1) * 512],
                            start=True,
                            stop=True,
                        )
                    return st

                def exp_kc(kc, st):
                    pt = ppool.tile([128, 2, 512], mmdt, tag="pt",
                                    name=f"pt{fc}_{qn}_{kc}")
                    if SKIPEXP:
                        if kc == 0:
                            nc.vector.memset(pt[:], 0.01)
                            pts["static"] = pt
                        pt = pts["static"]
                    else:
                        nc.scalar.activation(pt[:], st[:], AF.Exp,
                                             scale=1.0 / (8.0 * 1024.0))
                    pts[kc] = pt

                def attnv_kc(kc):
                    # h0 rotates over 2 accumulator banks (merged on DVE at
                    # normalize); h1 single-banked. Same-bank accumulating
                    # matmuls closer than ~850ns stall the PE.
                    dst0 = poA if kc % 2 == 0 else poB
                    nc.tensor.matmul(
                        dst0[:],
                        lhsT=v_sb[:, kc, h0 * 65:(h0 + 1) * 65],
                        rhs=pts[kc][:, 0, :],
                        start=(kc < 2),
                        stop=(kc >= NKC - 2),
                    )
                    nc.tensor.matmul(
                        po1[:],
                        lhsT=v_sb[:, kc, h1 * 65:(h1 + 1) * 65],
                        rhs=pts[kc][:, 1, :],
                        start=(kc == 0),
                        stop=(kc == NKC - 1),
                    )
                    del pts[kc]

                # software pipeline: scores(kc) || exp(kc) || attnV(kc-1),
                # K-projection pieces spread between iterations (front-
                # loaded so the DVE is free near the group boundary)
                st = scores_kc(0)
                exp_kc(0, st)
                for kc in range(1, NKC):
                    st = scores_kc(kc)
                    attnv_kc(kc - 1)
                    exp_kc(kc, st)
                    # 42 pieces spread over the 30 kc-slots of the 2 qn
                    # passes, front-loaded within each pass
                    npop = 0 if SKIPPROJ else 1
                    for _ in range(npop):
                        if pieces:
                            pieces.pop(0)()
                attnv_kc(NKC - 1)

                # psum-freeing copies on ACT (bounded lag): the next group's
                # attnV start=True matmuls WAW-wait these reads.
                ya = npool.tile([65, 512], f32, tag="ya",
                                name=f"ya{fc}_{qn}")
                nc.scalar.activation(ya[:], po1[:], AF.Identity)
                yb = npool.tile([65, 512], f32, tag="yb",
                                name=f"yb{fc}_{qn}")
                nc.scalar.activation(yb[:], poA[:], AF.Identity)
                ym = npool.tile([65, 512], f32, tag="ym",
                                name=f"ym{fc}_{qn}")
                nc.vector.tensor_add(ym[:], yb[:], poB[:])
                for i, (yy, hb) in enumerate(((ym, 0), (ya, 64))):
                    rz = npool.tile([1, 512], f32, tag=f"rz{i}", bufs=1,
                                    name=f"rz{i}_{fc}_{qn}")
                    nc.vector.reciprocal(rz[:], yy[64:65, :])
                    rzb = npool.tile([64, 512], f32, tag=f"rzb{i}", bufs=1,
                                     name=f"rzb{i}_{fc}_{qn}")
                    nc.gpsimd.partition_broadcast(rzb[:], rz[:])
                    nc.vector.tensor_mul(
                        y_sb[hb:hb + 64, fc, qn * 512:(qn + 1) * 512],
                        yy[0:64, :], rzb[:]
                    )

        def phase_qk_attn(do_attn, k_out, v_out):
            with tc.tile_pool(name="kstage", bufs=1) as kp:
                wo_sb = kp.tile([128, NFC, D], mmdt)
                # reload the gathered K'/V' halves (fc0 first: the first
                # attn pair waits on it)
                for fc in range(NFC):
                    for hh in range(2):
                        dma_rr(k_sb[:, fc, hh * NQ:(hh + 1) * NQ],
                               k_out[hh, fc])
                    if fc == 0:
                        for kc in range(NKC):
                            dma_rr(v_sb[:, kc, :], v_out[kc // 8, kc % 8])
                # prefetch output-projection weights into the attn window
                load_chunked(wo_sb, woT, NFC)
                with tc.tile_pool(name="qkpsum", bufs=1,
                                  space="PSUM") as qk_psum:
                    phase_qk_attn_inner(do_attn, qk_psum)
                # attn psum pools are closed; out projection runs here so
                # wo_sb (kstage) stays alive
                if PHASES >= 5:
                    phase_out(wo_sb)

        def phase_qk_attn_inner(do_attn, qk_psum):
            if True:
                if do_attn:
                    with tc.tile_pool(name="spsum", bufs=2,
                                      space="PSUM") as spool, \
                         tc.tile_pool(name="poApsum", bufs=1,
                                      space="PSUM") as poApool, \
                         tc.tile_pool(name="poBpsum", bufs=1,
                                      space="PSUM") as poBpool, \
                         tc.tile_pool(name="po1psum", bufs=1,
                                      space="PSUM") as po1pool, \
                         tc.tile_pool(name="ptpool", bufs=5) as ppool, \
                         tc.tile_pool(name="npool", bufs=2) as npool:
                        # pair 0's Q projection runs dense; pair p's attn
                        # spreads pair p+1's Q projection pieces.
                        for piece in make_q_pieces(0, qk_psum):
                            piece()
                        for fc in range(NFC):
                            if fc + 1 < NFC:
                                pieces = make_q_pieces(fc + 1, qk_psum)
                            else:
                                pieces = []
                            attn_pair(fc, spool, poApool, poBpool, po1pool,
                                      ppool, npool, pieces)
                            assert SKIPPROJ or not pieces, \
                                f"{len(pieces)} pieces left"
                else:
                    for fc in range(NFC):
                        for piece in make_q_pieces(fc, qk_psum):
                            piece()
                    nc.vector.memset(y_sb[:], 0.0)

        def phase_out(wo_sb):
            # 4 dmc x 2 rn rotating accumulator banks: per-bank WAW
            # distance is 8 matmuls, no accumulate stall.
            with tc.tile_pool(name="owork", bufs=3) as owork, \
                 tc.tile_pool(name="opsum2", bufs=8, space="PSUM") as ops2:
                for quad in range(2):
                    pss = {}
                    for d4 in range(4):
                        for rn in range(2):
                            pss[(d4, rn)] = ops2.tile(
                                [128, 512], f32, tag="ops",
                                name=f"ops{quad}_{d4}_{rn}")
                    for fc in range(NFC):
                        for d4 in range(4):
                            dmc = quad * 4 + d4
                            for rn in range(2):
                                nc.tensor.matmul(
                                    pss[(d4, rn)][:],
                                    lhsT=wo_sb[:, fc,
                                               dmc * 128:(dmc + 1) * 128],
                                    rhs=y_sb[:, fc, rn * 512:(rn + 1) * 512],
                                    start=(fc == 0),
                                    stop=(fc == NFC - 1),
                                )
                    for d4 in range(4):
                        dmc = quad * 4 + d4
                        ob = owork.tile([128, NQ], mmdt, tag="ob",
                                        name=f"ob{quad}_{d4}")
                        for rn in range(2):
                            nc.vector.tensor_scalar_add(
                                ob[:, rn * 512:(rn + 1) * 512],
                                pss[(d4, rn)][:],
                                bo_sb[:, dmc:dmc + 1])
                        eng = nc.sync if d4 % 2 == 0 else nc.gpsimd
                        eng.dma_start(outT[dmc * 128:(dmc + 1) * 128, :],
                                      ob[:])

        def all_phases():
            with tc.tile_pool(name="dram", bufs=1, space="DRAM") as dram:
                k_in = dram.tile([NFC, 128, 1024], mmdt, name="k_in")
                k_out = dram.tile([2, NFC, 128, 1024], mmdt, name="k_out")
                v_in = dram.tile([NKC // 2, 128, H * 65], mmdt, name="v_in")
                v_out = dram.tile([2, NKC // 2, 128, H * 65], mmdt,
                                  name="v_out")
                if PHASES >= 1:
                    phase_v(k_in, v_in)
                # pairwise sibling exchange of the K'/V' halves
                nc.gpsimd.collective_compute(
                    "AllGather", mybir.AluOpType.bypass,
                    replica_groups=[[0, 1], [2, 3], [4, 5], [6, 7]],
                    ins=[k_in.opt()], outs=[k_out.opt()],
                )
                nc.gpsimd.collective_compute(
                    "AllGather", mybir.AluOpType.bypass,
                    replica_groups=[[0, 1], [2, 3], [4, 5], [6, 7]],
                    ins=[v_in.opt()], outs=[v_out.opt()],
                )
                if PHASES >= 2:
                    phase_qk_attn(do_attn=(PHASES >= 4), k_out=k_out,
                                  v_out=v_out)
                else:
                    nc.vector.memset(y_sb[:], 0.0)
            if PHASES < 5:
                with tc.tile_pool(name="dummy", bufs=1) as dp:
                    zb = dp.tile([128, NQ], mmdt)
                    nc.vector.memset(zb[:], 0.0)
                    for dmc in range(NFC):
                        nc.sync.dma_start(outT[dmc * 128:(dmc + 1) * 128, :],
                                          zb[:])

        if KLOOP > 1:
            with tc.For_i(0, KLOOP, 1,
                          hint_engines=(mybir.EngineType.PE,
                                        mybir.EngineType.DVE,
                                        mybir.EngineType.Activation),
                          staggered_reset=True):
                all_phases()
        else:
            all_phases()

    nc.compile()
    return nc


def _rope_tables(positions):
    """cos/sin tables [128, len(positions)] for the permuted transposed
    layout: partition p (within a 2-head feature chunk), j = p % 64:
    j<16: freq j (cos, -sin); 32<=j<48: freq j-32 (cos, +sin); else (1, 0)."""
    inv_freq = 1.0 / (THETA ** (np.arange(0, ROT, 2, dtype=np.float64) / ROT))  # [16]
    t = np.asarray(positions, dtype=np.float64)
    ang = t[None, :] * inv_freq[:, None]  # [16, nt]
    c, s = np.cos(ang), np.sin(ang)
    cos_tab = np.ones((128, len(positions)), dtype=np.float64)
    sin_tab = np.zeros((128, len(positions)), dtype=np.float64)
    for h2 in (0, 64):
        cos_tab[h2:h2 + 16] = c
        cos_tab[h2 + 32:h2 + 48] = c
        # NEGATED sin coefficients (+s for evens, -s for odds): the device
        # reads the sin operand at the +-32 partner partition, where the
        # negated opposite-sign value equals the row's own coefficient.
        sin_tab[h2:h2 + 16] = s
        sin_tab[h2 + 32:h2 + 48] = -s
    return cos_tab.astype(np.float32), sin_tab.astype(np.float32)


def _head_perm():
    """Feature permutation applied to rows of Wq/Wk (and bq/bk): within each
    head's 64 outputs -> [evens(16), pass 32:48, odds(16), pass 48:64]."""
    out = np.empty(D, dtype=np.int64)
    for h in range(H):
        base = h * DK
        out[base:base + HALF] = base + np.arange(0, ROT, 2)
        out[base + HALF:base + ROT] = base + np.arange(ROT, ROT + HALF)
        out[base + ROT:base + ROT + HALF] = base + np.arange(1, ROT, 2)
        out[base + ROT + HALF:base + DK] = base + np.arange(ROT + HALF, DK)
    return out


def _prep_inputs(query, key, value, Wq, bq, Wk, bk, Wv, bv, Wo, bo,
                 mm_dtype_name="bfloat16"):
    import ml_dtypes

    np_mm = ml_dtypes.bfloat16 if mm_dtype_name == "bfloat16" else np.float32

    query = np.asarray(query, np.float32)
    key = np.asarray(key, np.float32)
    value = np.asarray(value, np.float32)
    Wq, bq = np.asarray(Wq, np.float32), np.asarray(bq, np.float32)
    Wk, bk = np.asarray(Wk, np.float32), np.asarray(bk, np.float32)
    Wv, bv = np.asarray(Wv, np.float32), np.asarray(bv, np.float32)
    Wo, bo = np.asarray(Wo, np.float32), np.asarray(bo, np.float32)

    perm = _head_perm()
    Wq_p, bq_p = Wq[perm], bq[perm]
    Wk_p, bk_p = Wk[perm], bk[perm]
    np_f8 = ml_dtypes.float8_e4m3

    def _pairs(a):  # [D, cols] -> [D//2, 2, cols] DoubleRow interleave
        d, cols = a.shape
        out = np.empty((d // 2, 2, cols), a.dtype)
        for dcp in range(d // 256):
            for j in (0, 1):
                out[dcp * 128:(dcp + 1) * 128, j, :] = \
                    a[(2 * dcp + j) * 128:(2 * dcp + j + 1) * 128, :]
        return out

    def _wflat(wt):  # [D, D] lhsT -> [128, NFC, 1024] = [p, fc, (c j m)]
        t = wt.reshape(4, 2, 128, H // 2, 128)  # [c, j, p, fc, m]
        return np.ascontiguousarray(
            t.transpose(2, 3, 0, 1, 4).reshape(128, H // 2, 1024))

    # Q/K path in fp8: weights prescaled by 32 (folded out in the exp scale)
    wq8T = _wflat((Wq_p.T * 32.0).astype(np_f8))
    wk8T = _wflat((Wk_p.T * 32.0).astype(np_f8))
    bq_p = bq_p * 32.0
    bk_p = bk_p * 32.0
    woT = np.ascontiguousarray(Wo.T).astype(np_mm)

    # W_v' : [D, H*65] plus a broadcast bias/ones tile wvb [128, H*65]
    wvT = np.zeros((D, H * 65), np.float32)
    wvb = np.zeros((1, H * 65), np.float32)
    for h in range(H):
        cols = slice(h * 65, h * 65 + 64)
        wvT[:D, cols] = Wv[h * DK:(h + 1) * DK, :].T
        wvb[0, cols] = bv[h * DK:(h + 1) * DK]
        wvb[0, h * 65 + 64] = 1.0
    wvT = wvT.astype(np_mm)
    wvb_bc = np.ascontiguousarray(np.repeat(wvb, 128, axis=0)).astype(np_mm)

    cos_all, sin_all = _rope_tables(np.arange(N))

    in_maps = []
    for core in range(NCORES):
        b, qh = core // 2, core % 2
        rows = slice(qh * NQ, (qh + 1) * NQ)
        xq8T = np.ascontiguousarray(
            _pairs(query[b, rows, :].T.astype(np_f8)))
        xk8T = np.ascontiguousarray(
            _pairs(key[b, rows, :].T.astype(np_f8)))
        xvT = np.ascontiguousarray(value[b, rows, :].T).astype(np_mm)
        in_maps.append({
            "xq8T": xq8T,
            "xk8T": xk8T,
            "xvT": xvT,
            "wq8T": wq8T, "wk8T": wk8T, "wvT": wvT, "woT": woT,
            "wvb": wvb_bc,
            "bq_d": bq_p, "bk_d": bk_p, "bo_d": bo,
            "cosQ": np.ascontiguousarray(cos_all[:, rows]).astype(np_mm),
            "sinQ": np.ascontiguousarray(sin_all[:, rows]).astype(np_mm),
        })
    return in_maps


def kernel(query, key, value, Wq, bq, Wk, bk, Wv, bv, Wo, bo):
    from concourse import bass_utils

    mm_dtype_name = "bfloat16"
    if mm_dtype_name not in _PROGRAM_CACHE:
        _PROGRAM_CACHE[mm_dtype_name] = _build_program(mm_dtype_name)
    nc = _PROGRAM_CACHE[mm_dtype_name]

    in_maps = _prep_inputs(query, key, value, Wq, bq, Wk, bk, Wv, bv, Wo, bo,
                           mm_dtype_name)

    res = bass_utils.run_bass_kernel_spmd(
        nc, in_maps, core_ids=list(range(NCORES))
    )

    out = np.empty((B, N, D), np.float32)
    for core in range(NCORES):
        b, qh = core // 2, core % 2
        out[b, qh * NQ:(qh + 1) * NQ, :] = \
            res.results[core]["outT"].T.astype(np.float32)
    return out
